# revision 1
# baseline (speedup 1.0000x reference)
"""GCN layer (gather + scale + segment-sum + linear + relu) on 8 TRN2 cores.

Sharding: each core owns a contiguous range of 6250 dst nodes and processes
every edge pointing into that range (edge lists are grouped by dst on the
host — pure format work, like building a CSR). Cores are fully independent:
no collectives.

Device pipeline per core (single phase, gather-bound):
  1. The gather table is the raw input: 256 B row per PAIR of nodes
     [featE(48 f16) | featO(48) | degE | degO | pad], so gather indices
     (src>>1) fit in signed int16 and no scaled table is ever built in
     DRAM. Per 128-dst-node block, dma_gather (SWDGE -> 16 SDMA engines)
     pulls the per-edge rows into SBUF.
  2. Per-slot X' = feat * rsqrt(max(out_deg,1)) on DVE, using the degrees
     that ride in each gathered row; only the parity half each tile
     actually reads is scaled.
  3. Transposed one-hot segment-sum on TensorE: psT[feat, node] +=
     msg^T @ one-hot, so no PE transpose is needed before the linear.
     Main slots (rank<16 per (node, src-parity)) share one constant
     one-hot rhs; overflow edges use per-lane target ids expanded on DVE
     to one-hot masks via is_equal against an iota constant (2 B/lane
     instead of full masks in the block stream). A full-width level-2
     matmul opens/closes the PSUM accumulation (start/stop act on whole
     partition rows).
  4. h^T = psT * rsqrt(max(in_deg,1)) straight into SBUF (fused
     PSUM-drain + scale); zero-in-degree fallback only compiled when such
     nodes exist. Linear + biased relu run interleaved every 4 blocks and
     the transposed output is written out per 512-column chunk.
Host concatenates + transposes the 8 output slices.
"""

import numpy as np

N = 50000
E = 1600000
D = 48
NCORES = 8
NPC = 6250             # nodes per core
BLOCKS = 49            # node range padded to 49*128 = 6272
NPAD = BLOCKS * 128
PAIRS = 25088          # rows in the pair table (incl. zero rows)
ZPAIR = 25000          # an all-zero pair row used for padding slots
WMAIN = 16             # main slots per (node, parity)
GROUPS = 4             # 32-node groups per block
OVG_T = 1              # level-1 overflow tiles per group

_CACHE = {}


# ---------------------------------------------------------------------------
# Host-side preprocessing: dtype narrowing, edge grouping by dst, slot
# assignment, one-hot mask construction, layout reshapes. All value math
# (rsqrt, scaling, sums, linear) runs on device.
# ---------------------------------------------------------------------------

def _host_prep(features, src, dst):
    src = np.asarray(src).astype(np.int64)
    dst = np.asarray(dst).astype(np.int64)
    feats = np.asarray(features, dtype=np.float32)

    par = (src & 1).astype(np.int64)
    pair = (src >> 1).astype(np.int64)
    out_deg = np.bincount(src, minlength=N).astype(np.int32)
    in_deg = np.bincount(dst, minlength=N).astype(np.int32)

    # fp16 gather table: 256 B row per pair of nodes, feature halves plus the
    # raw out-degrees (small ints are exact in fp16) packed into the pad
    # bytes. The device gathers rows per edge and applies rsqrt(max(deg,1))
    # scaling per slot, so no scaled table is ever materialized in DRAM.
    xrow = np.zeros((PAIRS, 128), dtype=np.float16)
    xrow[: N // 2, 0:48] = feats[0::2]
    xrow[: N // 2, 48:96] = feats[1::2]
    xrow[: N // 2, 96] = out_deg[0::2]
    xrow[: N // 2, 97] = out_deg[1::2]
    xrow[N // 2:, 96:98] = 1.0  # synthetic pad rows: rsqrt(1) = 1
    # every gathered slot's used half has deg >= 1 (an edge implies its
    # source has out-degree >= 1), so the clamp is only needed if some
    # node has out-degree 0 yet is a neighbor via the other parity
    nzdeg = not bool((out_deg == 0).any())

    # rank of each edge within its (dst, parity) bucket
    key = dst * 2 + par
    sort2 = np.argsort(key, kind="stable")
    ks = key[sort2]
    runstart = np.r_[0, np.flatnonzero(np.diff(ks)) + 1]
    runid = np.zeros(E, np.int64)
    runid[runstart] = 1
    runid = np.cumsum(runid) - 1
    rank = np.empty(E, np.int64)
    rank[sort2] = np.arange(E) - runstart[runid]

    core = dst // NPC
    nl = dst - core * NPC
    block = nl // 128
    v = nl % 128
    g = v // 32

    # ---- main slots (rank < WMAIN) --------------------------------------
    # slot: tile T = par*16 + g*4 + rank//4, lane = (v%32)*4 + rank%4
    selm = rank < WMAIN
    Tm = par[selm] * WMAIN + g[selm] * 4 + rank[selm] // 4
    lanem = (v[selm] % 32) * 4 + rank[selm] % 4

    # ---- overflow (rank >= WMAIN): level-1 per (core,block,group) -------
    selo = ~selm
    okey = (core[selo] * BLOCKS + block[selo]) * GROUPS + g[selo]
    osort = np.argsort(okey, kind="stable")
    oks = okey[osort]
    orunstart = np.r_[0, np.flatnonzero(np.diff(oks)) + 1]
    orunid = np.zeros(len(oks), np.int64)
    orunid[orunstart] = 1
    orunid = np.cumsum(orunid) - 1
    q = np.empty(len(oks), np.int64)
    q[osort] = np.arange(len(oks)) - orunstart[orunid]

    lvl1 = q < OVG_T * 128
    # ---- level-2: leftovers per (core, block) ---------------------------
    sel2 = ~lvl1
    oidx = np.flatnonzero(selo)
    e2 = oidx[sel2]
    k2 = core[e2] * BLOCKS + block[e2]
    s2 = np.argsort(k2, kind="stable")
    k2s = k2[s2]
    if len(k2s):
        rs2 = np.r_[0, np.flatnonzero(np.diff(k2s)) + 1]
        rid2 = np.zeros(len(k2s), np.int64)
        rid2[rs2] = 1
        rid2 = np.cumsum(rid2) - 1
        q2 = np.empty(len(k2s), np.int64)
        q2[s2] = np.arange(len(k2s)) - rs2[rid2]
        OV2_T = max(1, int(np.ceil((q2.max() + 1) / 128)))
    else:
        q2 = np.zeros(0, np.int64)
        OV2_T = 1
    assert OV2_T <= 4, f"unexpectedly deep level-2 overflow: {OV2_T}"

    TILES = 2 * WMAIN + GROUPS * OVG_T + OV2_T
    NIDX = TILES * 128

    gidx = np.full((NCORES, BLOCKS, TILES, 128), ZPAIR, np.int32)
    # per-lane scatter targets for the overflow tiles (-1 = inactive); the
    # device expands them to one-hot matmul masks with an is_equal against
    # an iota constant, so only 2 B/lane/tile ride in the block stream
    tg1 = np.full((NCORES, BLOCKS, GROUPS * OVG_T, 2, 128), -1, np.float16)
    tg2 = np.full((NCORES, BLOCKS, OV2_T, 2, 128), -1, np.float16)

    gidx[core[selm], block[selm], Tm, lanem] = pair[selm]

    e1 = oidx[lvl1]
    t1 = q[lvl1] // 128
    lane1 = q[lvl1] % 128
    T1 = 2 * WMAIN + g[e1] * OVG_T + t1
    gidx[core[e1], block[e1], T1, lane1] = pair[e1]
    tg1[core[e1], block[e1], g[e1] * OVG_T + t1, par[e1], lane1] = v[e1] % 32

    t2 = q2 // 128
    lane2 = q2 % 128
    T2 = 2 * WMAIN + GROUPS * OVG_T + t2
    gidx[core[e2], block[e2], T2, lane2] = pair[e2]
    tg2[core[e2], block[e2], t2, par[e2], lane2] = v[e2]

    # Pad slots (unused main/overflow lanes) all point at ZPAIR, the all-zero
    # row, so every block gathers exactly NIDX valid rows and num_idxs_reg is
    # the compile-time constant NIDX — no per-block count registers.
    iso = bool((in_deg == 0).any())  # any zero-in-degree node anywhere
    per_core = []
    for c in range(NCORES):
        flat = gidx[c].reshape(BLOCKS, NIDX).astype(np.int16)
        wrapped = flat.reshape(BLOCKS, NIDX // 16, 16).transpose(0, 2, 1)
        gidx_w = np.broadcast_to(
            wrapped[:, None, :, :], (BLOCKS, 8, 16, NIDX // 16)
        ).reshape(BLOCKS, 128, NIDX // 16).copy()

        tg1_dev = np.ascontiguousarray(
            tg1[c].transpose(0, 3, 1, 2).reshape(
                BLOCKS, 128, GROUPS * OVG_T * 2))
        tg2_dev = np.ascontiguousarray(
            tg2[c].transpose(0, 3, 1, 2).reshape(BLOCKS, 128, OV2_T * 2))

        nlo = c * NPC
        ind = np.zeros(NPAD, np.uint8)
        assert in_deg.max() <= 255
        ind[:NPC] = in_deg[nlo:nlo + NPC]
        # transposed + replicated across the 48 feature partitions so the
        # device can compute cj in the [feat, node] layout the blend uses
        indegT_dev = np.ascontiguousarray(
            np.broadcast_to(ind[None, :], (D, NPAD)))

        blk = np.concatenate([
            gidx_w.view(np.uint8).reshape(BLOCKS, 128, -1),
            tg1_dev.view(np.uint8).reshape(BLOCKS, 128, -1),
            tg2_dev.view(np.uint8).reshape(BLOCKS, 128, -1),
        ], axis=2)
        entry = dict(blk=np.ascontiguousarray(blk), indegT=indegT_dev)
        if iso:
            fc = np.zeros((NPAD, D), np.float32)
            fc[:NPC] = feats[nlo:nlo + NPC]
            entry["featT"] = np.ascontiguousarray(fc.T)
        per_core.append(entry)

    meta = dict(OV2_T=OV2_T, TILES=TILES, NIDX=NIDX, iso=iso, nzdeg=nzdeg)
    return xrow, per_core, meta


# ---------------------------------------------------------------------------
# Device program
# ---------------------------------------------------------------------------

def _build_program(meta):
    import concourse.tile as tile
    from concourse import bacc, mybir

    OV2_T, TILES, NIDX = meta["OV2_T"], meta["TILES"], meta["NIDX"]
    iso = meta["iso"]
    f16 = mybir.dt.float16
    f32 = mybir.dt.float32
    i16 = mybir.dt.int16
    AF = mybir.ActivationFunctionType
    OP = mybir.AluOpType

    nc = bacc.Bacc("TRN2", target_bir_lowering=False, debug=False,
                   num_devices=NCORES, num_swdge_queues=2)

    xrow = nc.dram_tensor("xrow", [PAIRS, 128], f16, kind="ExternalInput").ap()
    GIB = (NIDX // 16) * 2
    NT1 = GROUPS * OVG_T * 2
    NT2 = OV2_T * 2
    MGB = NT1 * 2
    M2B = NT2 * 2
    BLKB = GIB + MGB + M2B
    u8 = mybir.dt.uint8
    blkD = nc.dram_tensor("blk", [BLOCKS, 128, BLKB], u8, kind="ExternalInput").ap()
    indegTD = nc.dram_tensor("indegT", [D, NPAD], u8, kind="ExternalInput").ap()
    wbD = nc.dram_tensor("wb", [D, D], f32, kind="ExternalInput").ap()
    biasD = nc.dram_tensor("bias", [D, 1], f32, kind="ExternalInput").ap()
    id32D = nc.dram_tensor("id32", [128, GROUPS * 32], f16, kind="ExternalInput").ap()
    iotaD = nc.dram_tensor("iota", [128, NT1 * 32 + NT2 * 128], f16,
                           kind="ExternalInput").ap()
    if iso:
        featTD = nc.dram_tensor("featT", [D, NPAD], f32,
                                kind="ExternalInput").ap()
    outD = nc.dram_tensor("out", [D, NPAD], f16, kind="ExternalOutput").ap()

    with tile.TileContext(nc) as tc:
        with tc.tile_pool(name="const", bufs=1) as cpool, \
             tc.tile_pool(name="big", bufs=1) as bigpool:

            wb_s = cpool.tile([D, D], f32, tag="wb")
            nc.scalar.dma_start(out=wb_s[:], in_=wbD)
            bias_s = cpool.tile([D, 1], f32, tag="bias")
            nc.scalar.dma_start(out=bias_s[:], in_=biasD)
            id32 = cpool.tile([128, GROUPS * 32], f16, tag="id32")
            nc.scalar.dma_start(out=id32[:], in_=id32D)
            iota = cpool.tile([128, NT1 * 32 + NT2 * 128], f16, tag="iota")
            nc.scalar.dma_start(out=iota[:], in_=iotaD)
            iot1 = iota[:, 0:NT1 * 32].rearrange("p (t j) -> p t j", t=NT1)
            iot2 = iota[:, NT1 * 32:].rearrange("p (t j) -> p t j", t=NT2)

            # cj in the transposed [feat, node] layout the blend uses;
            # in-degree arrives pre-replicated across the 48 feature rows
            cjT = bigpool.tile([D, NPAD], f32, tag="cjT")
            ind_s = bigpool.tile([D, NPAD], u8, tag="indT")
            nc.scalar.dma_start(out=ind_s[:], in_=indegTD)
            nc.vector.tensor_scalar_max(cjT[:], ind_s[:], 1.0)
            nc.scalar.activation(cjT[:], cjT[:], AF.Sqrt)
            nc.vector.reciprocal(cjT[:], cjT[:])
            if iso:
                mask = bigpool.tile([D, NPAD], f32, tag="mask")
                nc.vector.tensor_scalar(mask[:], ind_s[:], 0.0, None,
                                        OP.is_gt)
                nc.vector.tensor_mul(cjT[:], cjT[:], mask[:])
                featT_s = bigpool.tile([D, NPAD], f32, tag="featT")
                nc.sync.dma_start(out=featT_s[:], in_=featTD)
                fbT = bigpool.tile([D, NPAD], f32, tag="fbT")
                nc.vector.tensor_scalar(mask[:], mask[:], -1.0, 1.0,
                                        OP.mult, OP.add)  # 1 - mask
                nc.vector.tensor_mul(fbT[:], featT_s[:], mask[:])

            hT1 = bigpool.tile([D, BLOCKS * 128], f32, tag="hT1")

            # ---- gather + transposed segment-sum over 49 blocks ----------
            with tc.tile_pool(name="blk", bufs=8) as blkpool, \
                 tc.tile_pool(name="msg", bufs=5) as msgpool, \
                 tc.tile_pool(name="sm", bufs=4) as smpool, \
                 tc.tile_pool(name="ps", bufs=4, space="PSUM") as pspool, \
                 tc.tile_pool(name="aux", bufs=3, space="PSUM") as auxpool:

                CH = 512
                outT = bigpool.tile([D, BLOCKS * 128], f16, tag="outT")

                for b in range(BLOCKS):
                    blkt = blkpool.tile([128, BLKB], u8, tag="blkt")
                    # block 0 rides the Pool queue (25 ns decode vs SP's
                    # 565 ns) to shorten the first-gather latency chain
                    eng = nc.gpsimd if b == 0 else nc.sync
                    eng.dma_start(out=blkt[:], in_=blkD[b])
                    gi = blkt[:, 0:GIB].bitcast(i16)
                    tg1 = blkt[:, GIB:GIB + MGB].bitcast(f16)
                    tg2 = blkt[:, GIB + MGB:BLKB].bitcast(f16)

                    msg = msgpool.tile([128, TILES, 128], f16, tag="msg")
                    nc.gpsimd.dma_gather(
                        out_ap=msg[:],
                        in_ap=xrow,
                        idxs_ap=gi,
                        num_idxs=NIDX,
                        num_idxs_reg=NIDX,
                        elem_size=128,
                        queue_num=b % 2,
                        single_packet=False,
                    )

                    # per-slot X' = feat * rsqrt(max(out_deg, 1)); degrees
                    # for both parities ride in cols 96:98 of each row.
                    # Main tiles are parity-pure, so only the half the
                    # matmuls actually read gets scaled; overflow tiles
                    # (both halves read) get both.
                    sc32 = smpool.tile([128, TILES, 2], f32, tag="sc32")
                    if meta["nzdeg"]:
                        nc.scalar.activation(sc32[:], msg[:, :, 96:98],
                                             AF.Sqrt)
                    else:
                        nc.vector.tensor_scalar_max(
                            sc32[:], msg[:, :, 96:98], 1.0)
                        nc.scalar.activation(sc32[:], sc32[:], AF.Sqrt)
                    sc = smpool.tile([128, TILES, 2], f16, tag="sc")
                    with nc.allow_low_precision("rsqrt scale, values <= 1"):
                        nc.vector.reciprocal(sc[:], sc32[:])

                    # expand per-lane targets into one-hot matmul masks
                    oh1 = smpool.tile([128, NT1, 32], f16, tag="oh1")
                    nc.vector.tensor_tensor(
                        oh1[:], iot1,
                        tg1.unsqueeze(2).to_broadcast([128, NT1, 32]),
                        OP.is_equal)
                    oh2 = smpool.tile([128, NT2, 128], f16, tag="oh2")
                    nc.vector.tensor_tensor(
                        oh2[:], iot2,
                        tg2.unsqueeze(2).to_broadcast([128, NT2, 128]),
                        OP.is_equal)
                    W2 = 2 * WMAIN
                    # overflow tiles first: the level-2 opener matmul only
                    # needs those, so the PE stream starts while the main
                    # halves are still being scaled
                    for t0, t1, p, c0 in ((W2, TILES, 0, 0),
                                          (W2, TILES, 1, 48),
                                          (0, WMAIN, 0, 0),
                                          (WMAIN, W2, 1, 48)):
                        nt = t1 - t0
                        nc.vector.tensor_tensor(
                            msg[:, t0:t1, c0:c0 + 48],
                            msg[:, t0:t1, c0:c0 + 48],
                            sc[:, t0:t1, p:p + 1].to_broadcast([128, nt, 48]),
                            OP.mult)

                    # transposed segment-sum: psT[feat, node] += msg^T @ 1hot
                    # PSUM start/stop act on whole partition rows, so the
                    # full-width level-2 matmuls open (start) and close
                    # (stop) the accumulation; everything else accumulates
                    # in between on 32-col slices.
                    psT = pspool.tile([D, 128], f32, tag="psT")
                    T2a = 2 * WMAIN + GROUPS * OVG_T
                    nc.tensor.matmul(
                        psT[:, :], lhsT=msg[:, T2a, 0:D],
                        rhs=oh2[:, 0, :],
                        start=True, stop=False, skip_group_check=True)
                    for p, c0 in ((0, 0), (1, 48)):
                        for gg in range(GROUPS):
                            for j in range(4):
                                T = p * WMAIN + gg * 4 + j
                                nc.tensor.matmul(
                                    psT[:, 32 * gg:32 * (gg + 1)],
                                    lhsT=msg[:, T, c0:c0 + D],
                                    rhs=id32[:, 32 * gg:32 * (gg + 1)],
                                    start=False, stop=False,
                                    skip_group_check=True)
                    for gg in range(GROUPS):
                        for t in range(OVG_T):
                            T = 2 * WMAIN + gg * OVG_T + t
                            for p, c0 in ((0, 0), (1, 48)):
                                nc.tensor.matmul(
                                    psT[:, 32 * gg:32 * (gg + 1)],
                                    lhsT=msg[:, T, c0:c0 + D],
                                    rhs=oh1[:, (gg * OVG_T + t) * 2 + p, :],
                                    start=False, stop=False,
                                    skip_group_check=True)
                    for t in range(OV2_T):
                        T = T2a + t
                        for pi, (p, c0) in enumerate(((0, 0), (1, 48))):
                            if t == 0 and pi == 0:
                                continue  # issued above as the opener
                            last = (t == OV2_T - 1) and (pi == 1)
                            nc.tensor.matmul(
                                psT[:, :],
                                lhsT=msg[:, T, c0:c0 + D],
                                rhs=oh2[:, t * 2 + p, :],
                                start=False, stop=last, skip_group_check=True)

                    # h^T = agg^T * cj (+ feat^T on zero-in-degree nodes)
                    bsl = slice(b * 128, (b + 1) * 128)
                    if iso:
                        th = smpool.tile([D, 128], f32, tag="th")
                        nc.vector.tensor_tensor(th[:], psT[:], cjT[:, bsl],
                                                OP.mult)
                        nc.vector.tensor_add(hT1[:, bsl], th[:],
                                             fbT[:, bsl])
                    else:
                        nc.vector.tensor_tensor(hT1[:, bsl], psT[:],
                                                cjT[:, bsl], OP.mult)

                    # ---- linear + relu, interleaved per 4 blocks ---------
                    if b % 4 == 3 or b == BLOCKS - 1:
                        lo = (b // 4) * CH
                        hi = (b + 1) * 128
                        po = auxpool.tile([D, CH], f32, tag="aux")
                        nc.tensor.matmul(po[:, 0:hi - lo], lhsT=wb_s[:],
                                         rhs=hT1[:, lo:hi], start=True,
                                         stop=True)
                        nc.scalar.activation(outT[:, lo:hi], po[:, 0:hi - lo],
                                             AF.Relu, bias=bias_s[:, 0:1])
                        nc.sync.dma_start(out=outD[:, lo:hi],
                                          in_=outT[:, lo:hi])

                # output stays transposed [48, NPAD], written per chunk
                # inside the loop; host transposes back

    nc.compile()
    return nc


# ---------------------------------------------------------------------------
# Entry point
# ---------------------------------------------------------------------------

def kernel(features, src, dst, W, b):
    from concourse.bass_utils import run_bass_kernel_spmd

    xrow, per_core, meta = _host_prep(features, src, dst)

    key = (meta["OV2_T"], meta["iso"], meta["nzdeg"])
    if key not in _CACHE:
        _CACHE[key] = _build_program(meta)
    nc = _CACHE[key]

    Wb = np.ascontiguousarray(np.asarray(W, np.float32).T)
    bias = np.ascontiguousarray(np.asarray(b, np.float32)[:, None])
    id32 = np.zeros((128, GROUPS * 32), np.float16)
    lanes = np.arange(128)
    for gg in range(GROUPS):
        id32[lanes, gg * 32 + lanes // 4] = 1.0
    nt1 = GROUPS * OVG_T * 2
    nt2 = meta["OV2_T"] * 2
    iorow = np.concatenate([
        np.tile(np.arange(32, dtype=np.float16), nt1),
        np.tile(np.arange(128, dtype=np.float16), nt2)])
    iota = np.ascontiguousarray(
        np.broadcast_to(iorow[None, :], (128, iorow.size)))

    in_maps = []
    for c in range(NCORES):
        pc = per_core[c]
        m = {
            "xrow": xrow, "blk": pc["blk"], "indegT": pc["indegT"],
            "wb": Wb, "bias": bias, "id32": id32, "iota": iota,
        }
        if meta["iso"]:
            m["featT"] = pc["featT"]
        in_maps.append(m)

    res = run_bass_kernel_spmd(nc, in_maps, core_ids=list(range(NCORES)))
    globals()["LAST_RESULTS"] = res
    out = np.concatenate(
        [res.results[c]["out"][:, :NPC].T for c in range(NCORES)], axis=0)
    return np.ascontiguousarray(out, dtype=np.float32)



# revision 8
# speedup vs baseline: 1.2373x; 1.2373x over previous
"""GCN layer (scale + segment-sum + linear + relu) on 8 TRN2 cores.

Sharding: each core owns a contiguous range of 6250 dst nodes and processes
every edge pointing into that range (same dst-sharding as a gather design),
but the per-edge data movement is INVERTED: instead of gathering a 256 B
row per edge (22.8 ns/descriptor), the kernel streams the feature table in
sequentially once, scales it by rsqrt(out_deg) on DVE, and scatter-adds the
96 B message of each edge into a DRAM accumulator via dma_scatter_add
(8.5 ns/descriptor — elem 48 fp16 with a 256 B row stride).

The SDMA CCE read-modify-write is NOT atomic across engines, so two adds to
the same accumulator row inside one scatter instruction (or in two
concurrently draining instructions) lose updates. The host therefore packs
edges into batches where (a) the sources form one slot per table position
over a contiguous slice (holes encoded as -1 indices, skipped positionally
by the Q7 ucode), and (b) the dst rows are DISTINCT — via per-batch maximum
bipartite matching. Batches rotate over 3 independent accumulators so their
WAW chains pipeline without racing; the tail transpose-reads the three
accumulators back, sums them, applies rsqrt(in_deg), and runs the 48x48
linear + relu exactly like the tail of the gather design.

The feature table is permuted per core by in-core degree (descending) so a
round-r batch touches only a prefix of positions; all value math (rsqrt,
scaling, sums, linear) runs on device — host work is index/format only.
One program is compiled for all 8 cores: batch shapes are the max over
cores, and per-core variation lives entirely in the index data (-1 pads).
"""

import numpy as np

N = 50000
E = 1600000
D = 48
NCORES = 8
NPC = 6250             # dst nodes per core
BLOCKS = 49            # dst range padded to 49*128 = 6272
NPAD = BLOCKS * 128
TCH = 391              # feature-table chunks: 391*128 = 50048 positions
NTAB = TCH * 128
SLCH = 36              # slice = 36 chunks = 4608 positions
SL = SLCH * 128
NACC = 3               # rotating DRAM accumulators

_CACHE = {}


# ---------------------------------------------------------------------------
# Host-side preprocessing: edge batching (bipartite matching per batch so
# every dst row within a scatter instruction is unique), node permutation,
# index wrapping. All value math runs on device.
# ---------------------------------------------------------------------------

NBINS = 8              # matched batches per slice in the main sweep
THETA = 0.5            # natural-density gate for emitting a (slice, bin)


def _build_core_batches(src_c, dstl_c):
    """Pack one core's edges into race-free scatter batches.

    Main sweep: per 36-chunk slice, up to NBINS maximum matchings; each
    matched set becomes one batch (distinct dst rows by construction).
    Everything left over goes to the annex: each remaining edge gets a
    fresh dedicated table slot (its source row duplicated), grouped by
    rank-within-dst so every annex batch is dense and dst-distinct.

    Returns (pi, main: {(s, b): (lo, hi, idxarr)}, annex_groups:
    [dst arrays], annex_srcs: [src-node arrays]) with slot arrays indexed
    from the slice base.
    """
    from scipy.sparse import csr_matrix
    from scipy.sparse.csgraph import maximum_bipartite_matching

    degc = np.bincount(src_c, minlength=N)
    pi = np.argsort(-degc, kind="stable")
    pos = np.empty(N, np.int64)
    pos[pi] = np.arange(N)

    p = pos[src_c]
    order = np.argsort(p, kind="stable")
    ps = p[order]
    ds = dstl_c[order].astype(np.int64)
    src_o = src_c[order]
    ne = len(ps)
    consumed = np.zeros(ne, bool)
    cnt = np.bincount(ps, minlength=NTAB)
    indptr = np.zeros(NTAB + 1, np.int64)
    indptr[1:] = np.cumsum(cnt)
    degpos = cnt.copy()

    main = {}
    for s in range(-(-NTAB // SL)):
        base = s * SL
        hi = min(base + SL, NTAB)
        e0, e1 = indptr[base], indptr[hi]
        if e0 == e1:
            continue
        for b in range(NBINS):
            nat = int((degpos[base:hi] > b).sum())
            if nat < THETA * (hi - base):
                break
            sub = np.arange(e0, e1)[~consumed[e0:e1]]
            if sub.size == 0:
                break
            rows = ps[sub] - base
            cols = ds[sub]
            g = csr_matrix((np.ones(sub.size, np.int8), (rows, cols)),
                           shape=(hi - base, NPC))
            m = maximum_bipartite_matching(g, perm_type="column")
            mr = np.flatnonzero(m >= 0)
            if mr.size == 0:
                break
            okey = rows * NPC + cols
            osort = np.argsort(okey, kind="stable")
            want = mr * NPC + m[mr]
            j = np.searchsorted(okey[osort], want)
            pick = sub[osort[j]]
            consumed[pick] = True
            arr = np.full(hi - base, -1, np.int16)
            arr[mr] = m[mr].astype(np.int16)
            main[(s, b)] = (int(mr[0]), int(mr[-1]) + 1, arr)

    rem = np.flatnonzero(~consumed)
    annex_groups = []
    annex_srcs = []
    if rem.size:
        rd = ds[rem]
        rs = src_o[rem]
        o2 = np.argsort(rd, kind="stable")
        rds = rd[o2]
        rss = rs[o2]
        runstart = np.r_[0, np.flatnonzero(np.diff(rds)) + 1]
        runid = np.zeros(rem.size, np.int64)
        runid[runstart] = 1
        runid = np.cumsum(runid) - 1
        rank = np.arange(rem.size) - runstart[runid]
        for k in range(int(rank.max()) + 1):
            selk = rank == k
            annex_groups.append(rds[selk].astype(np.int16))
            annex_srcs.append(rss[selk])
    return pi, main, annex_groups, annex_srcs


def _host_prep(features, src, dst):
    src = np.asarray(src).astype(np.int64)
    dst = np.asarray(dst).astype(np.int64)
    feats = np.asarray(features, dtype=np.float32)
    feats16 = feats.astype(np.float16)

    out_deg = np.bincount(src, minlength=N).astype(np.int64)
    in_deg = np.bincount(dst, minlength=N).astype(np.int64)
    iso = bool((in_deg == 0).any())

    core = dst // NPC
    raw = []
    for c in range(NCORES):
        sel = core == c
        raw.append(_build_core_batches(src[sel], dst[sel] - c * NPC))

    # ---- uniform cross-core plan -------------------------------------
    # main batches: union of (slice, bin) keys; per batch the chunk start
    # and length cover every core's matched span
    keys = sorted({k for _, m, _, _ in raw for k in m})
    mplan = []
    for (s, b) in keys:
        base = s * SL
        lo = SL
        hiv = 0
        for _, m, _, _ in raw:
            ent = m.get((s, b))
            if ent is not None:
                lo = min(lo, ent[0])
                hiv = max(hiv, ent[1])
        ca = s * SLCH + lo // 128
        L = -(-(hiv - (lo // 128) * 128) // 16) * 16
        mplan.append((s, b, ca, L))

    # annex: rank-group k starts at a uniform chunk; its slot count is the
    # max over cores (padded to 16)
    ngroups = max(len(g) for _, _, g, _ in raw)
    gch = []
    for k in range(ngroups):
        gmax = max((len(g[k]) if k < len(g) else 0) for _, _, g, _ in raw)
        gch.append(-(-gmax // 128))
    astart = []
    ach = TCH
    for k in range(ngroups):
        astart.append(ach)
        ach += gch[k]
    tch2 = ach
    aplan = []
    for k in range(ngroups):
        gmax = max((len(g[k]) if k < len(g) else 0) for _, _, g, _ in raw)
        aplan.append((astart[k], -(-gmax // 16) * 16))

    # combined device plan: (chunk_start, padded_len) per batch
    plan = tuple([(ca, L) for _, _, ca, L in mplan] + aplan)

    per_core = []
    for c in range(NCORES):
        pi, main, groups, gsrcs = raw[c]
        featp = np.zeros((tch2 * 128, D), np.float16)
        featp[:N] = feats16[pi]
        degp = np.zeros(tch2 * 128, np.float16)
        degp[:N] = out_deg[pi]
        for k in range(len(groups)):
            a0 = astart[k] * 128
            featp[a0:a0 + len(gsrcs[k])] = feats16[gsrcs[k]]
            degp[a0:a0 + len(gsrcs[k])] = out_deg[gsrcs[k]]
        featp_dev = np.ascontiguousarray(
            featp.reshape(tch2, 128, D).transpose(1, 0, 2))
        degp_dev = np.ascontiguousarray(degp.reshape(tch2, 128).T)

        bufs = []
        for (s, b, ca, L) in mplan:
            buf = np.full(L, -1, np.int16)
            ent = main.get((s, b))
            if ent is not None:
                off = ca - s * SLCH
                seg = ent[2][off * 128: off * 128 + L]
                buf[:len(seg)] = seg
            bufs.append(buf)
        for k, (cak, L) in enumerate(aplan):
            buf = np.full(L, -1, np.int16)
            if k < len(groups):
                buf[:len(groups[k])] = groups[k]
            bufs.append(buf)
        # the Q7 scatter ucode wedges on long trailing -1 runs across many
        # instructions; point trailing pads at the accumulator's unused pad
        # rows [NPC, NPAD) instead (their garbage is discarded by the tail)
        for buf in bufs:
            v = np.flatnonzero(buf >= 0)
            last = int(v[-1]) + 1 if v.size else 0
            t = np.arange(len(buf) - last)
            buf[last:] = (NPC + t % (NPAD - NPC)).astype(np.int16)
        chunks = []
        for buf in bufs:
            L = len(buf)
            wrapped = buf.reshape(L // 16, 16).T
            chunks.append(np.broadcast_to(
                wrapped[None, :, :], (8, 16, L // 16)).reshape(128, L // 16))
        idxcat = np.ascontiguousarray(np.concatenate(chunks, axis=1))

        nlo = c * NPC
        ind = np.zeros(NPAD, np.uint8)
        ind[:NPC] = np.minimum(in_deg[nlo:nlo + NPC], 255)
        indegT_dev = np.ascontiguousarray(
            np.broadcast_to(ind[None, :], (D, NPAD)))

        entry = dict(featp=featp_dev, degp=degp_dev, idxcat=idxcat,
                     indegT=indegT_dev)
        if iso:
            fc = np.zeros((NPAD, D), np.float32)
            fc[:NPC] = feats[nlo:nlo + NPC]
            entry["featT"] = np.ascontiguousarray(fc.T)
        per_core.append(entry)

    assert in_deg.max() <= 255
    tot16 = sum(L // 16 for _, L in plan)
    meta = dict(plan=plan, tot16=tot16, iso=iso, tch2=tch2)
    return per_core, meta


# ---------------------------------------------------------------------------
# Device program
# ---------------------------------------------------------------------------

def _build_program(meta):
    import concourse.tile as tile
    from concourse import bacc, mybir

    plan = meta["plan"]
    tot16 = meta["tot16"]
    iso = meta["iso"]
    tch2 = meta["tch2"]
    f16 = mybir.dt.float16
    f32 = mybir.dt.float32
    i16 = mybir.dt.int16
    u8 = mybir.dt.uint8
    AF = mybir.ActivationFunctionType
    OP = mybir.AluOpType

    nc = bacc.Bacc("TRN2", target_bir_lowering=False, debug=False,
                   num_devices=NCORES, num_swdge_queues=2)

    featpD = nc.dram_tensor("featp", [128, tch2, D], f16,
                            kind="ExternalInput").ap()
    degpD = nc.dram_tensor("degp", [128, tch2], f16, kind="ExternalInput").ap()
    idxcatD = nc.dram_tensor("idxcat", [128, tot16], i16,
                             kind="ExternalInput").ap()
    indegTD = nc.dram_tensor("indegT", [D, NPAD], u8, kind="ExternalInput").ap()
    wbD = nc.dram_tensor("wb", [D, D], f32, kind="ExternalInput").ap()
    biasD = nc.dram_tensor("bias", [D, 1], f32, kind="ExternalInput").ap()
    if iso:
        featTD = nc.dram_tensor("featT", [D, NPAD], f32,
                                kind="ExternalInput").ap()
    accD = [nc.dram_tensor(f"acc{a}", [NPAD, 128], f16,
                           kind="ExternalOutput").ap()
            for a in range(NACC)]
    outD = nc.dram_tensor("out", [D, NPAD], f16, kind="ExternalOutput").ap()

    with tile.TileContext(nc) as tc:
        with tc.tile_pool(name="const", bufs=1) as cpool, \
             tc.tile_pool(name="big", bufs=1) as bigpool:

            wb_s = cpool.tile([D, D], f32, tag="wb")
            nc.scalar.dma_start(out=wb_s[:], in_=wbD)
            bias_s = cpool.tile([D, 1], f32, tag="bias")
            nc.scalar.dma_start(out=bias_s[:], in_=biasD)

            # zero the accumulators first so the scatters can start early
            z = bigpool.tile([128, BLOCKS, 128], f16, tag="z")
            nc.vector.memset(z[:], 0.0)
            for a in range(NACC):
                nc.sync.dma_start(
                    out=accD[a].rearrange("(b p) c -> p b c", p=128),
                    in_=z[:])

            # per-src scale ci = rsqrt(max(out_deg, 1)); degrees are exact
            # small ints in fp16
            deg_s = bigpool.tile([128, tch2], f16, tag="deg")
            nc.scalar.dma_start(out=deg_s[:], in_=degpD)
            ci32 = bigpool.tile([128, tch2], f32, tag="ci32")
            nc.vector.tensor_scalar_max(ci32[:], deg_s[:], 1.0)
            nc.scalar.activation(ci32[:], ci32[:], AF.Sqrt)
            ci = bigpool.tile([128, tch2], f16, tag="ci")
            with nc.allow_low_precision("rsqrt scale, values <= 1"):
                nc.vector.reciprocal(ci[:], ci32[:])

            # feature table: load + scale per 44-chunk slice so the first
            # scatter batches only wait on slice 0
            featp_s = bigpool.tile([128, tch2, D], f16, tag="featp")
            LCH = 44
            nslice = -(-tch2 // LCH)
            for s in range(nslice):
                ca = s * LCH
                kch = min(LCH, tch2 - ca)
                nc.sync.dma_start(out=featp_s[:, ca:ca + kch, :],
                                  in_=featpD[:, ca:ca + kch, :])
                nc.vector.tensor_tensor(
                    featp_s[:, ca:ca + kch, :],
                    featp_s[:, ca:ca + kch, :],
                    ci[:, ca:ca + kch].unsqueeze(2).to_broadcast(
                        [128, kch, D]),
                    OP.mult)

            # race-free scatter-adds: every batch has distinct dst rows and
            # consecutive batches hit different accumulators; per-batch idx
            # tiles stream through a small pool instead of sitting in SBUF
            with tc.tile_pool(name="idxp", bufs=6) as idxpool:
                off = 0
                for i, (ca, L) in enumerate(plan):
                    a = i % NACC
                    K = -(-L // 128)
                    idx_s = idxpool.tile([128, L // 16], i16, tag="idx")
                    nc.scalar.dma_start(out=idx_s[:],
                                        in_=idxcatD[:, off:off + L // 16])
                    nc.gpsimd.dma_scatter_add(
                        accD[a][:, 0:D],
                        featp_s[:, ca:ca + K, :],
                        idx_s[:],
                        L,
                        L,
                        D,
                        elem_step=128,
                        queue_num=i % 2,
                        single_packet=False,
                    )
                    off += L // 16

            # ---- tail: read back, combine, cj scale, linear + relu -------
            cjT = bigpool.tile([D, NPAD], f32, tag="cjT")
            ind_s = bigpool.tile([D, NPAD], u8, tag="indT")
            nc.scalar.dma_start(out=ind_s[:], in_=indegTD)
            nc.vector.tensor_scalar_max(cjT[:], ind_s[:], 1.0)
            nc.scalar.activation(cjT[:], cjT[:], AF.Sqrt)
            nc.vector.reciprocal(cjT[:], cjT[:])
            if iso:
                mask = bigpool.tile([D, NPAD], f32, tag="mask")
                nc.vector.tensor_scalar(mask[:], ind_s[:], 0.0, None,
                                        OP.is_gt)
                nc.vector.tensor_mul(cjT[:], cjT[:], mask[:])
                featT_s = bigpool.tile([D, NPAD], f32, tag="featT")
                nc.sync.dma_start(out=featT_s[:], in_=featTD)
                fbT = bigpool.tile([D, NPAD], f32, tag="fbT")
                nc.vector.tensor_scalar(mask[:], mask[:], -1.0, 1.0,
                                        OP.mult, OP.add)  # 1 - mask
                nc.vector.tensor_mul(fbT[:], featT_s[:], mask[:])

            hs = bigpool.tile([D, NPAD], f32, tag="hs")
            outT = bigpool.tile([D, NPAD], f16, tag="outT")
            with tc.tile_pool(name="htp", bufs=2) as htpool:
                ht0 = htpool.tile([128, NPAD], f16, tag="ht")
                nc.sync.dma_start(out=ht0[:], in_=accD[0], transpose=True)
                ht1 = htpool.tile([128, NPAD], f16, tag="ht")
                nc.sync.dma_start(out=ht1[:], in_=accD[1], transpose=True)
                nc.vector.tensor_tensor(hs[:], ht0[0:D, :], ht1[0:D, :],
                                        OP.add)
                ht2 = htpool.tile([128, NPAD], f16, tag="ht")
                nc.sync.dma_start(out=ht2[:], in_=accD[2], transpose=True)
                nc.vector.tensor_tensor(hs[:], hs[:], ht2[0:D, :], OP.add)
            if iso:
                nc.vector.tensor_mul(hs[:], hs[:], cjT[:])
                nc.vector.tensor_add(hs[:], hs[:], fbT[:])
            else:
                nc.vector.tensor_mul(hs[:], hs[:], cjT[:])

            CH = 512
            with tc.tile_pool(name="ps", bufs=2, space="PSUM") as pspool:
                for lo in range(0, NPAD, CH):
                    hi = min(lo + CH, NPAD)
                    po = pspool.tile([D, CH], f32, tag="po")
                    nc.tensor.matmul(po[:, 0:hi - lo], lhsT=wb_s[:],
                                     rhs=hs[:, lo:hi], start=True, stop=True)
                    nc.scalar.activation(outT[:, lo:hi], po[:, 0:hi - lo],
                                         AF.Relu, bias=bias_s[:, 0:1])
                    nc.sync.dma_start(out=outD[:, lo:hi], in_=outT[:, lo:hi])

    nc.compile()
    return nc


# ---------------------------------------------------------------------------
# Entry point
# ---------------------------------------------------------------------------

def kernel(features, src, dst, W, b):
    from concourse.bass_utils import run_bass_kernel_spmd

    per_core, meta = _host_prep(features, src, dst)

    key = (meta["plan"], meta["iso"], meta["tch2"])
    if key not in _CACHE:
        _CACHE[key] = _build_program(meta)
    nc = _CACHE[key]

    Wb = np.ascontiguousarray(np.asarray(W, np.float32).T)
    bias = np.ascontiguousarray(np.asarray(b, np.float32)[:, None])

    in_maps = []
    for c in range(NCORES):
        pc = per_core[c]
        m = {
            "featp": pc["featp"], "degp": pc["degp"],
            "idxcat": pc["idxcat"], "indegT": pc["indegT"],
            "wb": Wb, "bias": bias,
        }
        if meta["iso"]:
            m["featT"] = pc["featT"]
        in_maps.append(m)

    res = run_bass_kernel_spmd(nc, in_maps, core_ids=list(range(NCORES)))
    globals()["LAST_RESULTS"] = res
    out = np.concatenate(
        [res.results[c]["out"][:, :NPC].T for c in range(NCORES)], axis=0)
    return np.ascontiguousarray(out, dtype=np.float32)


# revision 13
# speedup vs baseline: 1.3872x; 1.1212x over previous
"""GCN layer (scale + segment-sum + linear + relu) on 8 TRN2 cores.

Sharding: each core owns a contiguous range of 6250 dst nodes and processes
every edge pointing into that range (same dst-sharding as a gather design),
but the per-edge data movement is INVERTED: instead of gathering a 256 B
row per edge (22.8 ns/descriptor), the kernel streams the feature table in
sequentially once, scales it by rsqrt(out_deg) on DVE, and scatter-adds the
96 B message of each edge into a DRAM accumulator via dma_scatter_add
(8.5 ns/descriptor — elem 48 fp16 with a 256 B row stride).

The SDMA CCE read-modify-write is NOT atomic across engines, so two adds to
the same accumulator row inside one scatter instruction (or in two
concurrently draining instructions) lose updates. The host therefore packs
edges into batches where (a) the sources form one slot per table position
over a contiguous slice (holes encoded as -1 indices, skipped positionally
by the Q7 ucode), and (b) the dst rows are DISTINCT — via per-batch maximum
bipartite matching. Batches rotate over 3 independent accumulators so their
WAW chains pipeline without racing; the tail transpose-reads the three
accumulators back, sums them, applies rsqrt(in_deg), and runs the 48x48
linear + relu exactly like the tail of the gather design.

The feature table is permuted per core by in-core degree (descending) so a
round-r batch touches only a prefix of positions; all value math (rsqrt,
scaling, sums, linear) runs on device — host work is index/format only.
One program is compiled for all 8 cores: batch shapes are the max over
cores, and per-core variation lives entirely in the index data (-1 pads).
"""

import numpy as np

N = 50000
E = 1600000
D = 48
NCORES = 8
NPC = 6250             # dst nodes per core
BLOCKS = 49            # dst range padded to 49*128 = 6272
NPAD = BLOCKS * 128
TCH = 391              # feature-table chunks: 391*128 = 50048 positions
NTAB = TCH * 128
SLCH = 36              # slice = 36 chunks = 4608 positions
SL = SLCH * 128
NACC = 3               # rotating DRAM accumulators

_CACHE = {}


# ---------------------------------------------------------------------------
# Host-side preprocessing: edge batching (bipartite matching per batch so
# every dst row within a scatter instruction is unique), node permutation,
# index wrapping. All value math runs on device.
# ---------------------------------------------------------------------------

NBINS = 8              # matched batches per slice in the main sweep
THETA = 0.5            # natural-density gate for emitting a (slice, bin)


def _build_core_batches(src_c, dstl_c):
    """Pack one core's edges into race-free scatter batches.

    Main sweep: per 36-chunk slice, up to NBINS maximum matchings; each
    matched set becomes one batch (distinct dst rows by construction).
    Everything left over goes to the annex: each remaining edge gets a
    fresh dedicated table slot (its source row duplicated), grouped by
    rank-within-dst so every annex batch is dense and dst-distinct.

    Returns (pi, main: {(s, b): (lo, hi, idxarr)}, annex_groups:
    [dst arrays], annex_srcs: [src-node arrays]) with slot arrays indexed
    from the slice base.
    """
    from scipy.sparse import csr_matrix
    from scipy.sparse.csgraph import maximum_bipartite_matching

    degc = np.bincount(src_c, minlength=N)
    pi = np.argsort(-degc, kind="stable")
    pos = np.empty(N, np.int64)
    pos[pi] = np.arange(N)

    p = pos[src_c]
    order = np.argsort(p, kind="stable")
    ps = p[order]
    ds = dstl_c[order].astype(np.int64)
    src_o = src_c[order]
    ne = len(ps)
    consumed = np.zeros(ne, bool)
    cnt = np.bincount(ps, minlength=NTAB)
    indptr = np.zeros(NTAB + 1, np.int64)
    indptr[1:] = np.cumsum(cnt)
    degpos = cnt.copy()

    main = {}
    for s in range(-(-NTAB // SL)):
        base = s * SL
        hi = min(base + SL, NTAB)
        e0, e1 = indptr[base], indptr[hi]
        if e0 == e1:
            continue
        for b in range(NBINS):
            nat = int((degpos[base:hi] > b).sum())
            if nat < THETA * (hi - base):
                break
            sub = np.arange(e0, e1)[~consumed[e0:e1]]
            if sub.size == 0:
                break
            rows = ps[sub] - base
            cols = ds[sub]
            g = csr_matrix((np.ones(sub.size, np.int8), (rows, cols)),
                           shape=(hi - base, NPC))
            m = maximum_bipartite_matching(g, perm_type="column")
            mr = np.flatnonzero(m >= 0)
            if mr.size == 0:
                break
            okey = rows * NPC + cols
            osort = np.argsort(okey, kind="stable")
            want = mr * NPC + m[mr]
            j = np.searchsorted(okey[osort], want)
            pick = sub[osort[j]]
            consumed[pick] = True
            arr = np.full(hi - base, -1, np.int16)
            arr[mr] = m[mr].astype(np.int16)
            main[(s, b)] = (int(mr[0]), int(mr[-1]) + 1, arr)

    rem = np.flatnonzero(~consumed)
    annex_groups = []
    annex_srcs = []
    if rem.size:
        rd = ds[rem]
        rs = src_o[rem]
        o2 = np.argsort(rd, kind="stable")
        rds = rd[o2]
        rss = rs[o2]
        runstart = np.r_[0, np.flatnonzero(np.diff(rds)) + 1]
        runid = np.zeros(rem.size, np.int64)
        runid[runstart] = 1
        runid = np.cumsum(runid) - 1
        rank = np.arange(rem.size) - runstart[runid]
        for k in range(int(rank.max()) + 1):
            selk = rank == k
            annex_groups.append(rds[selk].astype(np.int16))
            annex_srcs.append(rss[selk])
    return pi, main, annex_groups, annex_srcs


def _host_prep(features, src, dst):
    src = np.asarray(src).astype(np.int64)
    dst = np.asarray(dst).astype(np.int64)
    feats = np.asarray(features, dtype=np.float32)
    feats16 = feats.astype(np.float16)

    out_deg = np.bincount(src, minlength=N).astype(np.int64)
    in_deg = np.bincount(dst, minlength=N).astype(np.int64)
    iso = bool((in_deg == 0).any())

    core = dst // NPC
    raw = []
    for c in range(NCORES):
        sel = core == c
        raw.append(_build_core_batches(src[sel], dst[sel] - c * NPC))

    # ---- uniform cross-core plan -------------------------------------
    # main batches: union of (slice, bin) keys; per batch the chunk start
    # and length cover every core's matched span
    keys = sorted({k for _, m, _, _ in raw for k in m})
    mplan = []
    for (s, b) in keys:
        base = s * SL
        lo = SL
        hiv = 0
        for _, m, _, _ in raw:
            ent = m.get((s, b))
            if ent is not None:
                lo = min(lo, ent[0])
                hiv = max(hiv, ent[1])
        ca = s * SLCH + lo // 128
        L = -(-(hiv - (lo // 128) * 128) // 16) * 16
        mplan.append((s, b, ca, L))

    # annex: rank-group k starts at a uniform chunk; its slot count is the
    # max over cores (padded to 16)
    ngroups = max(len(g) for _, _, g, _ in raw)
    gch = []
    for k in range(ngroups):
        gmax = max((len(g[k]) if k < len(g) else 0) for _, _, g, _ in raw)
        gch.append(-(-gmax // 128))
    astart = []
    ach = TCH
    for k in range(ngroups):
        astart.append(ach)
        ach += gch[k]
    tch2 = ach
    aplan = []
    for k in range(ngroups):
        gmax = max((len(g[k]) if k < len(g) else 0) for _, _, g, _ in raw)
        aplan.append((astart[k], -(-gmax // 16) * 16))

    # combined device plan: (chunk_start, padded_len) per batch, largest
    # last so the final batches of each accumulator overlap the transpose
    # readbacks of the others
    plan_list = [(ca, L) for _, _, ca, L in mplan] + aplan
    order = sorted(range(len(plan_list)), key=lambda i: plan_list[i][1])
    plan = tuple(plan_list[i] for i in order)

    per_core = []
    for c in range(NCORES):
        pi, main, groups, gsrcs = raw[c]
        featp = np.zeros((tch2 * 128, D), np.float16)
        featp[:N] = feats16[pi]
        degp = np.zeros(tch2 * 128, np.float16)
        degp[:N] = out_deg[pi]
        for k in range(len(groups)):
            a0 = astart[k] * 128
            featp[a0:a0 + len(gsrcs[k])] = feats16[gsrcs[k]]
            degp[a0:a0 + len(gsrcs[k])] = out_deg[gsrcs[k]]
        featp_dev = np.ascontiguousarray(
            featp.reshape(tch2, 128, D).transpose(1, 0, 2))
        degp_dev = np.ascontiguousarray(degp.reshape(tch2, 128).T)

        bufs = []
        for (s, b, ca, L) in mplan:
            buf = np.full(L, -1, np.int16)
            ent = main.get((s, b))
            if ent is not None:
                off = ca - s * SLCH
                seg = ent[2][off * 128: off * 128 + L]
                buf[:len(seg)] = seg
            bufs.append(buf)
        for k, (cak, L) in enumerate(aplan):
            buf = np.full(L, -1, np.int16)
            if k < len(groups):
                buf[:len(groups[k])] = groups[k]
            bufs.append(buf)
        # the Q7 scatter ucode wedges on long trailing -1 runs across many
        # instructions; point trailing pads at the accumulator's unused pad
        # rows [NPC, NPAD) instead (their garbage is discarded by the tail)
        for buf in bufs:
            v = np.flatnonzero(buf >= 0)
            last = int(v[-1]) + 1 if v.size else 0
            t = np.arange(len(buf) - last)
            buf[last:] = (NPC + t % (NPAD - NPC)).astype(np.int16)
        bufs = [bufs[i] for i in order]
        chunks = []
        for buf in bufs:
            L = len(buf)
            wrapped = buf.reshape(L // 16, 16).T
            chunks.append(np.broadcast_to(
                wrapped[None, :, :], (8, 16, L // 16)).reshape(128, L // 16))
        idxcat = np.ascontiguousarray(np.concatenate(chunks, axis=1))

        nlo = c * NPC
        ind = np.zeros(NPAD, np.uint8)
        ind[:NPC] = np.minimum(in_deg[nlo:nlo + NPC], 255)
        indegT_dev = np.ascontiguousarray(
            np.broadcast_to(ind[None, :], (D, NPAD)))

        entry = dict(featp=featp_dev, degp=degp_dev, idxcat=idxcat,
                     indegT=indegT_dev)
        if iso:
            fc = np.zeros((NPAD, D), np.float32)
            fc[:NPC] = feats[nlo:nlo + NPC]
            entry["featT"] = np.ascontiguousarray(fc.T)
        per_core.append(entry)

    assert in_deg.max() <= 255
    tot16 = sum(L // 16 for _, L in plan)
    meta = dict(plan=plan, tot16=tot16, iso=iso, tch2=tch2)
    return per_core, meta


# ---------------------------------------------------------------------------
# Device program
# ---------------------------------------------------------------------------

def _build_program(meta):
    import concourse.tile as tile
    from concourse import bacc, mybir

    plan = meta["plan"]
    tot16 = meta["tot16"]
    iso = meta["iso"]
    tch2 = meta["tch2"]
    f16 = mybir.dt.float16
    f32 = mybir.dt.float32
    i16 = mybir.dt.int16
    u8 = mybir.dt.uint8
    AF = mybir.ActivationFunctionType
    OP = mybir.AluOpType

    nc = bacc.Bacc("TRN2", target_bir_lowering=False, debug=False,
                   num_devices=NCORES, num_swdge_queues=2)

    featpD = nc.dram_tensor("featp", [128, tch2, D], f16,
                            kind="ExternalInput").ap()
    degpD = nc.dram_tensor("degp", [128, tch2], f16, kind="ExternalInput").ap()
    idxcatD = nc.dram_tensor("idxcat", [128, tot16], i16,
                             kind="ExternalInput").ap()
    indegTD = nc.dram_tensor("indegT", [D, NPAD], u8, kind="ExternalInput").ap()
    wbD = nc.dram_tensor("wb", [D, D], f16, kind="ExternalInput").ap()
    biasD = nc.dram_tensor("bias", [D, 1], f32, kind="ExternalInput").ap()
    if iso:
        featTD = nc.dram_tensor("featT", [D, NPAD], f32,
                                kind="ExternalInput").ap()
    accD = [nc.dram_tensor(f"acc{a}", [NPAD, 128], f16,
                           kind="ExternalOutput").ap()
            for a in range(NACC)]
    outD = nc.dram_tensor("out", [D, NPAD], f16, kind="ExternalOutput").ap()

    with tile.TileContext(nc) as tc:
        with tc.tile_pool(name="const", bufs=1) as cpool, \
             tc.tile_pool(name="big", bufs=1) as bigpool:

            wb_s = cpool.tile([D, D], f16, tag="wb")
            nc.scalar.dma_start(out=wb_s[:], in_=wbD)
            bias_s = cpool.tile([D, 1], f32, tag="bias")
            nc.scalar.dma_start(out=bias_s[:], in_=biasD)

            # zero the accumulators first so the scatters can start early
            z = bigpool.tile([128, BLOCKS, 128], f16, tag="z")
            nc.vector.memset(z[:], 0.0)
            for a in range(NACC):
                nc.sync.dma_start(
                    out=accD[a].rearrange("(b p) c -> p b c", p=128),
                    in_=z[:])

            # per-src scale ci = rsqrt(max(out_deg, 1)); degrees are exact
            # small ints in fp16
            deg_s = bigpool.tile([128, tch2], f16, tag="deg")
            nc.scalar.dma_start(out=deg_s[:], in_=degpD)
            ci = bigpool.tile([128, tch2], f16, tag="ci")

            # feature table: load + scale per 44-chunk slice so the first
            # scatter batches only wait on slice 0
            featp_s = bigpool.tile([128, tch2, D], f16, tag="featp")
            LCH = 44
            nslice = -(-tch2 // LCH)
            with tc.tile_pool(name="cip", bufs=3) as cipool:
                for s in range(nslice):
                    ca = s * LCH
                    kch = min(LCH, tch2 - ca)
                    nc.sync.dma_start(out=featp_s[:, ca:ca + kch, :],
                                      in_=featpD[:, ca:ca + kch, :])
                    c32 = cipool.tile([128, LCH], f32, tag="c32")
                    nc.vector.tensor_scalar_max(c32[:, 0:kch],
                                                deg_s[:, ca:ca + kch], 1.0)
                    nc.scalar.activation(c32[:, 0:kch], c32[:, 0:kch],
                                         AF.Sqrt)
                    with nc.allow_low_precision("rsqrt scale, values <= 1"):
                        nc.vector.reciprocal(ci[:, ca:ca + kch], c32[:, 0:kch])
                    nc.vector.tensor_tensor(
                        featp_s[:, ca:ca + kch, :],
                        featp_s[:, ca:ca + kch, :],
                        ci[:, ca:ca + kch].unsqueeze(2).to_broadcast(
                            [128, kch, D]),
                        OP.mult)

            # resident idx data, split into 4 tiles so early batches only
            # wait on the first load
            ngrp = 4
            goff = [0]
            gsz = []
            per = -(-len(plan) // ngrp)
            bnd = []
            o = 0
            for gi in range(ngrp):
                lo_b = gi * per
                hi_b = min((gi + 1) * per, len(plan))
                w = sum(L // 16 for _, L in plan[lo_b:hi_b])
                bnd.append((lo_b, hi_b, o))
                gsz.append(w)
                o += w
                goff.append(o)
            idx_tiles = []
            for gi in range(ngrp):
                t = bigpool.tile([128, max(gsz[gi], 1)], i16, tag=f"idx{gi}")
                if gsz[gi]:
                    nc.sync.dma_start(
                        out=t[:],
                        in_=idxcatD[:, goff[gi]:goff[gi] + gsz[gi]])
                idx_tiles.append(t)

            # cj = rsqrt(max(in_deg, 1)) in fp16, prepared during the
            # scatter phase (DVE is idle then)
            cjT = bigpool.tile([D, NPAD], f16, tag="cjT")
            ind_s = bigpool.tile([D, NPAD], u8, tag="indT")
            nc.scalar.dma_start(out=ind_s[:], in_=indegTD)
            cj32 = bigpool.tile([D, NPAD], f32, tag="cj32")
            nc.vector.tensor_scalar_max(cj32[:], ind_s[:], 1.0)
            nc.scalar.activation(cj32[:], cj32[:], AF.Sqrt)
            with nc.allow_low_precision("rsqrt scale, values <= 1"):
                nc.vector.reciprocal(cjT[:], cj32[:])
            if iso:
                mask = bigpool.tile([D, NPAD], f16, tag="mask")
                nc.vector.tensor_scalar(mask[:], ind_s[:], 0.0, None,
                                        OP.is_gt)
                nc.vector.tensor_mul(cjT[:], cjT[:], mask[:])
                featT_s = bigpool.tile([D, NPAD], f32, tag="featT")
                nc.sync.dma_start(out=featT_s[:], in_=featTD)
                fbT = bigpool.tile([D, NPAD], f32, tag="fbT")
                nc.vector.tensor_scalar(mask[:], mask[:], -1.0, 1.0,
                                        OP.mult, OP.add)  # 1 - mask
                nc.vector.tensor_mul(fbT[:], featT_s[:], mask[:])

            # race-free scatter-adds: every batch has distinct dst rows and
            # consecutive batches hit different accumulators
            for i, (ca, L) in enumerate(plan):
                a = i % NACC
                K = -(-L // 128)
                gi = min(i // per, ngrp - 1)
                lo_b, hi_b, obase = bnd[gi]
                loc = sum(LL // 16 for _, LL in plan[lo_b:i])
                nc.gpsimd.dma_scatter_add(
                    accD[a][:, 0:D],
                    featp_s[:, ca:ca + K, :],
                    idx_tiles[gi][:, loc:loc + L // 16],
                    L,
                    L,
                    D,
                    elem_step=128,
                    queue_num=i % 2,
                    single_packet=False,
                )

            # ---- tail: read back, combine, cj scale, linear + relu -------
            hs = bigpool.tile([D, NPAD], f16, tag="hs")
            outT = bigpool.tile([D, NPAD], f16, tag="outT")
            with tc.tile_pool(name="htp", bufs=2) as htpool:
                ht0 = htpool.tile([128, NPAD], f16, tag="ht")
                nc.sync.dma_start(out=ht0[:], in_=accD[0], transpose=True)
                ht1 = htpool.tile([128, NPAD], f16, tag="ht")
                nc.sync.dma_start(out=ht1[:], in_=accD[1], transpose=True)
                nc.vector.tensor_tensor(hs[:], ht0[0:D, :], ht1[0:D, :],
                                        OP.add)
                ht2 = htpool.tile([128, NPAD], f16, tag="ht")
                nc.sync.dma_start(out=ht2[:], in_=accD[2], transpose=True)
                nc.vector.tensor_tensor(hs[:], hs[:], ht2[0:D, :], OP.add)
            if iso:
                nc.vector.tensor_mul(hs[:], hs[:], cjT[:])
                nc.vector.tensor_add(hs[:], hs[:], fbT[:])
            else:
                nc.vector.tensor_mul(hs[:], hs[:], cjT[:])

            CH = 512
            with tc.tile_pool(name="ps", bufs=2, space="PSUM") as pspool:
                for lo in range(0, NPAD, CH):
                    hi = min(lo + CH, NPAD)
                    po = pspool.tile([D, CH], f32, tag="po")
                    nc.tensor.matmul(po[:, 0:hi - lo], lhsT=wb_s[:],
                                     rhs=hs[:, lo:hi], start=True, stop=True)
                    nc.scalar.activation(outT[:, lo:hi], po[:, 0:hi - lo],
                                         AF.Relu, bias=bias_s[:, 0:1])
                    nc.sync.dma_start(out=outD[:, lo:hi], in_=outT[:, lo:hi])

    nc.compile()
    return nc


# ---------------------------------------------------------------------------
# Entry point
# ---------------------------------------------------------------------------

def kernel(features, src, dst, W, b):
    from concourse.bass_utils import run_bass_kernel_spmd

    per_core, meta = _host_prep(features, src, dst)

    key = (meta["plan"], meta["iso"], meta["tch2"])
    if key not in _CACHE:
        _CACHE[key] = _build_program(meta)
    nc = _CACHE[key]

    Wb = np.ascontiguousarray(np.asarray(W, np.float32).T.astype(np.float16))
    bias = np.ascontiguousarray(np.asarray(b, np.float32)[:, None])

    in_maps = []
    for c in range(NCORES):
        pc = per_core[c]
        m = {
            "featp": pc["featp"], "degp": pc["degp"],
            "idxcat": pc["idxcat"], "indegT": pc["indegT"],
            "wb": Wb, "bias": bias,
        }
        if meta["iso"]:
            m["featT"] = pc["featT"]
        in_maps.append(m)

    res = run_bass_kernel_spmd(nc, in_maps, core_ids=list(range(NCORES)))
    globals()["LAST_RESULTS"] = res
    out = np.concatenate(
        [res.results[c]["out"][:, :NPC].T for c in range(NCORES)], axis=0)
    return np.ascontiguousarray(out, dtype=np.float32)
